# revision 1
# baseline (speedup 1.0000x reference)
"""DopDense forward: relu(x @ (w * mult) + b) on 8 trn2 NeuronCores.

Key algebra: w_new = w * mult (per-column scaling) commutes with the matmul,
so out = relu((x @ w) * mult[None, :] + b).  We compute y^T tiles (units on
partitions, batch on free axis) so the per-column mult/bias become
per-partition scale/bias of a fused Relu eviction (scalar-engine activation
or a 2-op vector tensor_scalar).

mult is computed on device: dd[j] = sum_i |w[i,d_j] - old[i,d_j]| (vector
engine), gating logic in j-space, then a multiplicative scatter to columns
as mult = (1 + L^T lfm1) * (1 + R^T rfm1) -- left/right target columns are
each unique, and the single collision (column 0) is handled exactly by the
product.  L/R are built on device from an iota constant via is_equal.

Sharding: data-parallel over the batch axis (8192 rows/core); w, dop state
replicated.  The big matmul runs in bf16, everything else fp32.  The kernel
is memory-bound (~25 MB/core), so DMA traffic is spread across the sync
HWDGE, scalar HWDGE and gpsimd SWDGE queues with few large DMAs.
"""

import numpy as np
import ml_dtypes


def _install_ntff_shim():
    """The trimmed antenv package in this image lacks axon_hooks, which
    concourse's trace=True path imports unconditionally.  Recreate the hook
    registry (and install the ctypes NTFF hook when available) so tracing
    works whether or not the caller enables it."""
    import sys
    import types
    try:
        import antenv
        import antenv.axon_hooks  # noqa: F401
        return
    except ImportError:
        pass
    try:
        import antenv
    except ImportError:
        return
    mod = types.ModuleType("antenv.axon_hooks")
    holder = [None]
    try:
        from trn_agent_boot.trn_boot import _ntff_profile_via_ctypes
        holder[0] = _ntff_profile_via_ctypes("/opt/axon/libaxon_pjrt.so")
    except Exception:
        pass
    mod.get_axon_ntff_profile_hook = lambda: holder[0]
    mod.set_axon_ntff_profile_hook = lambda h: holder.__setitem__(0, h)
    sys.modules["antenv.axon_hooks"] = mod
    antenv.axon_hooks = mod


_install_ntff_shim()

import concourse.bass as bass
import concourse.mybir as mybir
import concourse.tile as tile
from concourse import bacc
from concourse.bass_utils import run_bass_kernel_spmd

F32 = mybir.dt.float32
BF16 = mybir.dt.bfloat16
AF = mybir.ActivationFunctionType
ALU = mybir.AluOpType
BF16_NP = np.dtype(ml_dtypes.bfloat16)

N_CORES = 8
B = 65536
NIN = 512
UNITS = 512
N_DOP = 128
SHARD = B // N_CORES          # 8192 batch rows per core
W = 1024                      # batch window per psum tile (2 PSUM banks)
NWP = SHARD // W              # 8 windows per core
KC = NIN // 128               # 4 contraction chunks
CC = UNITS // 128             # 4 unit chunks
THRESHOLD = 0.0
REF_PERIOD = 2.0

# Static dopaminergic-column index math (mirrors reference.py exactly)
DOP_IDX = np.linspace(1, UNITS - 1, N_DOP, dtype=np.int32)
LEFT_OK = ~np.isin(DOP_IDX - 1, DOP_IDX)
RIGHT_OK = ~np.isin(DOP_IDX + 1, DOP_IDX)
LCOL = (DOP_IDX - 1) % UNITS
RCOL = (DOP_IDX + 1) % UNITS

LOK10 = LEFT_OK.astype(np.float32) * np.float32(10.0 / NIN)
ROK10 = RIGHT_OK.astype(np.float32) * np.float32(10.0 / NIN)

_CACHED_NC = None


def build_nc():
    global _CACHED_NC
    if _CACHED_NC is not None:
        return _CACHED_NC
    nc = bacc.Bacc("TRN2", target_bir_lowering=False, debug=False,
                   num_swdge_queues=2)

    xt = nc.dram_tensor("xt", [NWP, 128, KC * W], BF16, kind="ExternalInput")
    # w chunks packed as [128, (k*CC+c)*128 + m] (bf16, matmul stationary)
    wkb = nc.dram_tensor("wkb", [128, KC * CC * 128], BF16, kind="ExternalInput")
    # all aux inputs packed into one wide tensor (small-row DMAs are slow):
    # [:, 0:18] = per-partition vectors (lok10, rok10, indicator, batch_ctr,
    # b0..b3, lcol%128, rcol%128, Lchunkmask[4], Rchunkmask[4]),
    # [:, 18:146] = iota rows, [:, 146:658] = dop columns of w^T,
    # [:, 658:1170] = dop columns of old^T
    NV = 18
    auxs = nc.dram_tensor("auxs", [128, NV + 128], F32, kind="ExternalInput")
    auxb = nc.dram_tensor("auxb", [128, 2 * NIN], F32, kind="ExternalInput")
    yt = nc.dram_tensor("yt", [NWP, 128, CC * W], F32, kind="ExternalOutput")

    with tile.TileContext(nc) as tc:
        with (
            tc.tile_pool(name="const", bufs=1) as const,
            tc.tile_pool(name="aux", bufs=1) as aux,
            tc.tile_pool(name="xa", bufs=4) as xpool,
            tc.tile_pool(name="ob", bufs=5) as opool,
            tc.tile_pool(name="tmp", bufs=2) as tpool,
        ):
            # ---------- input DMAs: few, large, spread over 3 queues ----------
            # aux-critical inputs lead their queues (they gate mult, which
            # gates every eviction)
            wk_sb = const.tile([128, KC * CC * 128], BF16, tag="wk")
            nc.sync.dma_start(wk_sb[:], wkb[:])
            axs_sb = const.tile([128, NV + 128], F32, tag="axs")
            nc.sync.dma_start(axs_sb[:], auxs[:])
            axb_sb = const.tile([128, 2 * NIN], F32, tag="axb")
            nc.sync.dma_start(axb_sb[:, :NIN], auxb[:, :NIN])
            nc.scalar.dma_start(axb_sb[:, NIN:], auxb[:, NIN:])
            v_sb = axs_sb[:, 0:NV]
            io_sb = axs_sb[:, NV:NV + 128]
            wd_sb = axb_sb[:, 0:NIN]
            od_sb = axb_sb[:, NIN:2 * NIN]

            def wk_tile(k, c):
                i = k * CC + c
                return wk_sb[:, i * 128:(i + 1) * 128]

            # x windows: wp0 split for fast start; then alternate sync/gpsimd.
            xa_tiles = {}

            def load_xa(wp):
                xa = xpool.tile([128, KC * W], BF16, tag="xa")
                if wp == 0:
                    nc.sync.dma_start(xa[:, :2 * W], xt[0][:, :2 * W])
                    nc.scalar.dma_start(xa[:, 2 * W:], xt[0][:, 2 * W:])
                elif wp == 1:
                    nc.scalar.dma_start(xa[:, :2 * W], xt[1][:, :2 * W])
                    nc.gpsimd.dma_start(xa[:, 2 * W:], xt[1][:, 2 * W:])
                elif wp % 2 == 1 or wp == 6:
                    nc.gpsimd.dma_start(xa[:], xt[wp])
                else:
                    nc.sync.dma_start(xa[:], xt[wp])
                xa_tiles[wp] = xa

            for wp in range(3):
                load_xa(wp)

            # PE warm-up: the tensor engine is idle until x arrives; a burst
            # of dummy matmuls on scratch data lifts the HAM clock gate to
            # full speed before the real stream starts
            scr = const.tile([128, 512], BF16, tag="scr")
            nc.gpsimd.memset(scr[:], 0.0)

            # scatter masks from iota while waiting on wd/od:
            # Lmod[j, m] = 1 iff LCOL[j] % 128 == m (chunk selection happens
            # via the masked rhs columns in the scatter matmul)
            lmod = const.tile([128, 128], BF16, tag="lmod")
            nc.vector.tensor_scalar(lmod[:], io_sb, v_sb[:, 8:9],
                                    None, op0=ALU.is_equal)
            rmod = const.tile([128, 128], BF16, tag="rmod")
            nc.vector.tensor_scalar(rmod[:], io_sb, v_sb[:, 9:10],
                                    None, op0=ALU.is_equal)

            # ---------- aux compute: dd[j] = sum_i |w[i,d_j] - old[i,d_j]| ----
            dch = aux.tile([128, NIN], F32, tag="dch")
            nc.vector.tensor_tensor(dch[:], wd_sb, od_sb, op=ALU.subtract)
            dd = const.tile([128, 1], F32, tag="dd")
            nc.vector.tensor_reduce(
                dd[:], dch[:], axis=mybir.AxisListType.X, op=ALU.add,
                apply_absolute_value=True,
            )
            # active = (dd > THRESHOLD) & ((batch_ctr - indicator) > REF_PERIOD)
            t1 = const.tile([128, 1], F32, tag="t1")
            nc.vector.tensor_tensor(t1[:], v_sb[:, 3:4], v_sb[:, 2:3],
                                    op=ALU.subtract)
            c2 = const.tile([128, 1], F32, tag="c2")
            nc.vector.tensor_scalar(c2[:], t1[:], REF_PERIOD, None, op0=ALU.is_gt)
            c1 = const.tile([128, 1], F32, tag="c1")
            nc.vector.tensor_scalar(c1[:], dd[:], THRESHOLD, None, op0=ALU.is_gt)
            av = const.tile([128, 1], F32, tag="av")
            nc.vector.tensor_tensor(av[:], c1[:], c2[:], op=ALU.mult)
            da = const.tile([128, 1], F32, tag="da")
            nc.vector.tensor_tensor(da[:], dd[:], av[:], op=ALU.mult)
            lf1 = const.tile([128, 1], F32, tag="lf1")
            nc.vector.tensor_tensor(lf1[:], da[:], v_sb[:, 0:1], op=ALU.mult)
            rf1 = const.tile([128, 1], F32, tag="rf1")
            nc.vector.tensor_tensor(rf1[:], da[:], v_sb[:, 1:2], op=ALU.mult)

            # additive scatters (all 4 chunks in one matmul pair), then
            # mult = (1 + L^T lfm1) * (1 + R^T rfm1)
            lfc = const.tile([128, CC], BF16, tag="lfc")
            nc.vector.tensor_scalar(lfc[:], v_sb[:, 10:10 + CC], lf1[:],
                                    None, op0=ALU.mult)
            rfc = const.tile([128, CC], BF16, tag="rfc")
            nc.vector.tensor_scalar(rfc[:], v_sb[:, 14:14 + CC], rf1[:],
                                    None, op0=ALU.mult)
            with tc.tile_pool(name="psx", bufs=2, space="PSUM") as psaux:
                warm = psaux.tile([128, 512], F32, tag="auxps")
                for _ in range(12):
                    nc.tensor.matmul(warm[:], scr[:, :128], scr[:],
                                     start=True, stop=True)
                psl = psaux.tile([128, CC], F32, tag="auxps")
                nc.tensor.matmul(psl[:], lmod[:], lfc[:], start=True, stop=True)
                psr = psaux.tile([128, CC], F32, tag="auxps")
                nc.tensor.matmul(psr[:], rmod[:], rfc[:], start=True, stop=True)
                lsp = const.tile([128, CC], F32, tag="lsp")
                nc.vector.tensor_scalar(lsp[:], psl[:], 1.0, None, op0=ALU.add)
                rsp = const.tile([128, CC], F32, tag="rsp")
                nc.vector.tensor_scalar(rsp[:], psr[:], 1.0, None, op0=ALU.add)
                multm = const.tile([128, CC], F32, tag="multm")
                nc.vector.tensor_tensor(multm[:], lsp[:], rsp[:], op=ALU.mult)
            mult_sb = [multm[:, cc:cc + 1] for cc in range(CC)]

            # ---------- main: y^T = (w^T x^T) scaled+biased+relu ----------
            # Window pairs share each stationary weight across 4 matmuls.
            def evict_act(ps, ob, c):
                nc.scalar.activation(
                    ob[:, c * W:(c + 1) * W], ps[:], AF.Relu,
                    bias=v_sb[:, 4 + c:5 + c], scale=mult_sb[c])

            def evict_dve(ps, ob, c):
                tmp = tpool.tile([128, W], F32, tag="evt")
                nc.vector.tensor_scalar(
                    tmp[:], ps[:], mult_sb[c], v_sb[:, 4 + c:5 + c],
                    op0=ALU.mult, op1=ALU.add)
                nc.vector.tensor_scalar(
                    ob[:, c * W:(c + 1) * W], tmp[:], 0.0, None, op0=ALU.max)

            NSW = NWP // 2
            with tc.tile_pool(name="ps", bufs=4, space="PSUM") as pspool:
                for sw in range(NSW):
                    if sw + 1 < NSW:
                        load_xa(2 * (sw + 1))
                        load_xa(2 * (sw + 1) + 1)
                    wpa, wpb = 2 * sw, 2 * sw + 1
                    xaa, xab = xa_tiles[wpa], xa_tiles[wpb]
                    oba = opool.tile([128, CC * W], F32, tag="ob")
                    obb = opool.tile([128, CC * W], F32, tag="ob")
                    for c in range(CC):
                        psa = pspool.tile([128, W], F32, tag="mps")
                        psb = pspool.tile([128, W], F32, tag="mps")
                        korder = range(KC) if (sw * CC + c) % 2 == 0 \
                            else range(KC - 1, -1, -1)
                        for ki, k in enumerate(korder):
                            for ps, xa in ((psa, xaa), (psb, xab)):
                                for s in range(W // 512):
                                    nc.tensor.matmul(
                                        ps[:, s * 512:(s + 1) * 512],
                                        wk_tile(k, c),
                                        xa[:, k * W + s * 512: k * W + (s + 1) * 512],
                                        start=(ki == 0), stop=(ki == KC - 1),
                                    )
                        # 5 ACT + 3 DVE evictions per superwindow;
                        # last superwindow alternates for a parallel tail
                        last = sw == NSW - 1
                        if c < 2:
                            evict_act(psa, oba, c)
                            evict_act(psb, obb, c)
                        else:
                            # ACT+DVE per group: parallel engines bound the
                            # group's eviction latency at one DVE pass
                            evict_act(psa, oba, c)
                            evict_dve(psb, obb, c)
                        # drain each evicted c-chunk immediately; gpsimd
                        # (slow SWDGE drain) only carries mid-kernel pieces
                        if last and c >= 2:
                            h = c * W + W // 2
                            for wp_, ob_ in ((wpa, oba), (wpb, obb)):
                                nc.sync.dma_start(yt[wp_][:, c * W:h],
                                                  ob_[:, c * W:h])
                                nc.scalar.dma_start(yt[wp_][:, h:(c + 1) * W],
                                                    ob_[:, h:(c + 1) * W])
                            continue
                        if sw in (1, 2) and c < 2:
                            enga = engb = nc.gpsimd
                        elif sw == 0:
                            enga = engb = nc.scalar
                        else:
                            enga, engb = nc.scalar, nc.sync
                        enga.dma_start(yt[wpa][:, c * W:(c + 1) * W],
                                       oba[:, c * W:(c + 1) * W])
                        engb.dma_start(yt[wpb][:, c * W:(c + 1) * W],
                                       obb[:, c * W:(c + 1) * W])

    nc.compile()
    _CACHED_NC = nc
    return nc


LAST_RESULTS = None


def kernel(x, w, b, dop_weights_old, indicator, batch_ctr):
    global LAST_RESULTS
    x = np.asarray(x, dtype=np.float32)
    w = np.ascontiguousarray(np.asarray(w, dtype=np.float32))
    b_arr = np.asarray(b, dtype=np.float32)
    old = np.asarray(dop_weights_old, dtype=np.float32)
    ind = np.asarray(indicator, dtype=np.float32)
    bc_val = float(np.asarray(batch_ctr).item())

    nc = build_nc()

    # replicated (per-core identical) inputs; all reshapes/gathers are pure
    # data marshaling -- every arithmetic op happens on device
    wkb = np.ascontiguousarray(
        w.reshape(KC, 128, CC, 128).transpose(1, 0, 2, 3)
    ).reshape(128, KC * CC * 128).astype(BF16_NP)
    vcols = [LOK10, ROK10, ind.astype(np.float32),
             np.full(128, bc_val, np.float32)]
    vcols += [b_arr[c * 128:(c + 1) * 128] for c in range(CC)]
    vcols += [(LCOL % 128).astype(np.float32), (RCOL % 128).astype(np.float32)]
    vcols += [(LCOL // 128 == cc).astype(np.float32) for cc in range(CC)]
    vcols += [(RCOL // 128 == cc).astype(np.float32) for cc in range(CC)]
    vecs = np.stack(vcols, axis=1).astype(np.float32)
    iot = np.broadcast_to(np.arange(128, dtype=np.float32), (128, 128))
    auxs = np.ascontiguousarray(np.concatenate(
        [vecs, iot], axis=1, dtype=np.float32))
    auxb = np.ascontiguousarray(np.concatenate(
        [w.T[DOP_IDX], old.T[DOP_IDX]], axis=1, dtype=np.float32))

    common = dict(wkb=wkb, auxs=auxs, auxb=auxb)

    xbf = x.astype(BF16_NP)
    in_maps = []
    for i in range(N_CORES):
        xs = xbf[i * SHARD:(i + 1) * SHARD]          # [8192, 512]
        xtc = np.ascontiguousarray(
            xs.reshape(NWP, W, KC, 128).transpose(0, 3, 2, 1)
        ).reshape(NWP, 128, KC * W)
        in_maps.append(dict(common, xt=xtc))

    res = run_bass_kernel_spmd(nc, in_maps, core_ids=list(range(N_CORES)))
    LAST_RESULTS = res

    out = np.empty((B, UNITS), np.float32)
    for i in range(N_CORES):
        ytc = res.results[i]["yt"].reshape(NWP, 128, CC, W)
        out[i * SHARD:(i + 1) * SHARD] = (
            ytc.transpose(0, 3, 2, 1).reshape(SHARD, UNITS))
    return out



# revision 2
# speedup vs baseline: 1.0514x; 1.0514x over previous
"""DopDense forward: relu(x @ (w * mult) + b) on 8 trn2 NeuronCores.

Key algebra: w_new = w * mult (per-column scaling) commutes with the matmul,
so out = relu((x @ w) * mult[None, :] + b).  We compute y^T tiles (units on
partitions, batch on free axis) so the per-column mult/bias become
per-partition scale/bias of a fused Relu eviction (scalar-engine activation
or a 2-op vector tensor_scalar).

mult is computed on device: dd[j] = sum_i |w[i,d_j] - old[i,d_j]| (vector
engine), gating logic in j-space, then a multiplicative scatter to columns
as mult = (1 + L^T lfm1) * (1 + R^T rfm1) -- left/right target columns are
each unique, and the single collision (column 0) is handled exactly by the
product.  L/R are built on device from an iota constant via is_equal.

Sharding: data-parallel over the batch axis (8192 rows/core); w, dop state
replicated.  The big matmul runs in bf16 and the output is stored in bf16
(upconverted on host), so the kernel is tensor-engine bound (~55us of
matmul issue) with DMA (~18 MB/core) fully overlapped underneath.  DMA
priorities: the first window pair's x chunks lead the sync/gpsimd queues at
k-chunk granularity so the matmul stream starts as soon as data can land;
aux inputs lead the scalar queue so the column-scale vector is ready well
before the first eviction.
"""

import numpy as np
import ml_dtypes


def _install_ntff_shim():
    """The trimmed antenv package in this image lacks axon_hooks, which
    concourse's trace=True path imports unconditionally.  Recreate the hook
    registry (and install the ctypes NTFF hook when available) so tracing
    works whether or not the caller enables it."""
    import sys
    import types
    try:
        import antenv
        import antenv.axon_hooks  # noqa: F401
        return
    except ImportError:
        pass
    try:
        import antenv
    except ImportError:
        return
    mod = types.ModuleType("antenv.axon_hooks")
    holder = [None]
    try:
        from trn_agent_boot.trn_boot import _ntff_profile_via_ctypes
        holder[0] = _ntff_profile_via_ctypes("/opt/axon/libaxon_pjrt.so")
    except Exception:
        pass
    mod.get_axon_ntff_profile_hook = lambda: holder[0]
    mod.set_axon_ntff_profile_hook = lambda h: holder.__setitem__(0, h)
    sys.modules["antenv.axon_hooks"] = mod
    antenv.axon_hooks = mod


_install_ntff_shim()

import concourse.bass as bass
import concourse.mybir as mybir
import concourse.tile as tile
from concourse import bacc
from concourse.bass_utils import run_bass_kernel_spmd

F32 = mybir.dt.float32
BF16 = mybir.dt.bfloat16
AF = mybir.ActivationFunctionType
ALU = mybir.AluOpType
BF16_NP = np.dtype(ml_dtypes.bfloat16)

N_CORES = 8
B = 65536
NIN = 512
UNITS = 512
N_DOP = 128
SHARD = B // N_CORES          # 8192 batch rows per core
W = 1024                      # batch window per psum tile (2 PSUM banks)
NWP = SHARD // W              # 8 windows per core
KC = NIN // 128               # 4 contraction chunks
CC = UNITS // 128             # 4 unit chunks
THRESHOLD = 0.0
REF_PERIOD = 2.0
NWARM = 5                     # PE clock warm-up matmuls

# w chunk (k, c) visit order of the main loop: c outer, k fwd for even c and
# rev for odd c (shared x-chunk locality at c boundaries).  wkb is packed on
# host in this first-use order so a prefix DMA covers the c=0 chunks.
ORDER = []
for _c in range(CC):
    _ks = range(KC) if _c % 2 == 0 else range(KC - 1, -1, -1)
    for _k in _ks:
        ORDER.append((_k, _c))
WKPOS = {kc: i for i, kc in enumerate(ORDER)}

# Static dopaminergic-column index math (mirrors reference.py exactly)
DOP_IDX = np.linspace(1, UNITS - 1, N_DOP, dtype=np.int32)
LEFT_OK = ~np.isin(DOP_IDX - 1, DOP_IDX)
RIGHT_OK = ~np.isin(DOP_IDX + 1, DOP_IDX)
LCOL = (DOP_IDX - 1) % UNITS
RCOL = (DOP_IDX + 1) % UNITS

LOK10 = LEFT_OK.astype(np.float32) * np.float32(10.0 / NIN)
ROK10 = RIGHT_OK.astype(np.float32) * np.float32(10.0 / NIN)

_CACHED_NC = None


def build_nc():
    global _CACHED_NC
    if _CACHED_NC is not None:
        return _CACHED_NC
    nc = bacc.Bacc("TRN2", target_bir_lowering=False, debug=False,
                   num_swdge_queues=2)

    xt = nc.dram_tensor("xt", [NWP, 128, KC * W], BF16, kind="ExternalInput")
    # w chunks packed as [128, ORDER-position * 128 + m] (bf16, stationary)
    wkb = nc.dram_tensor("wkb", [128, KC * CC * 128], BF16, kind="ExternalInput")
    # aux inputs packed into one wide tensor (small-row DMAs are slow):
    # [:, 0:18] = per-partition vectors (lok10, rok10, indicator, batch_ctr,
    # b0..b3, lcol%128, rcol%128, Lchunkmask[4], Rchunkmask[4]),
    # [:, 18:146] = iota rows
    NV = 18
    auxs = nc.dram_tensor("auxs", [128, NV + 128], F32, kind="ExternalInput")
    # dop columns of w^T and old^T, in bf16 (the |w-old| reduction over 512
    # terms is insensitive to bf16 rounding; halves the critical aux DMA)
    auxb = nc.dram_tensor("auxb", [128, 2 * NIN], BF16, kind="ExternalInput")
    # output in bf16 (rel-err budget 2e-2; bf16 adds ~2e-3) -- halves the
    # dominant output DMA traffic. Host upconverts to fp32.
    yt = nc.dram_tensor("yt", [NWP, 128, CC * W], BF16, kind="ExternalOutput")

    with tile.TileContext(nc) as tc:
        with (
            tc.tile_pool(name="const", bufs=1) as const,
            tc.tile_pool(name="aux", bufs=1) as aux,
            tc.tile_pool(name="xa", bufs=4) as xpool,
            tc.tile_pool(name="ob", bufs=5) as opool,
            tc.tile_pool(name="tmp", bufs=2) as tpool,
        ):
            wk_sb = const.tile([128, KC * CC * 128], BF16, tag="wk")
            axs_sb = const.tile([128, NV + 128], F32, tag="axs")
            axb_sb = const.tile([128, 2 * NIN], BF16, tag="axb")
            xa_tiles = {}

            def load_xa(wp, fine=False):
                xa = xpool.tile([128, KC * W], BF16, tag="xa")
                eng = nc.sync if wp % 2 == 0 else nc.gpsimd
                if fine:
                    # k-chunk granularity: each chunk unblocks its matmuls
                    # as soon as it lands (deps are per-DMA-instruction)
                    for k in range(KC):
                        eng.dma_start(xa[:, k * W:(k + 1) * W],
                                      xt[wp][:, k * W:(k + 1) * W])
                else:
                    eng.dma_start(xa[:, :2 * W], xt[wp][:, :2 * W])
                    eng.dma_start(xa[:, 2 * W:], xt[wp][:, 2 * W:])
                xa_tiles[wp] = xa

            # ---------- input DMAs: emission order = per-queue priority ----
            # gpsimd queue: c=0 w chunks (gate the first matmuls), then x1
            nc.gpsimd.dma_start(wk_sb[:, :4 * 128], wkb[:, :4 * 128])
            # sync queue: x0 at k-chunk granularity
            load_xa(0, fine=True)
            # gpsimd: x1 k chunks (k3 rides on scalar to even the queues)
            xa1 = xpool.tile([128, KC * W], BF16, tag="xa")
            for k in range(KC - 1):
                nc.gpsimd.dma_start(xa1[:, k * W:(k + 1) * W],
                                    xt[1][:, k * W:(k + 1) * W])
            xa_tiles[1] = xa1
            # scalar queue: aux inputs (gate mult -> every eviction), x1k3,
            # then the remaining w chunks (first needed at c=1, ~mid pair 0)
            nc.scalar.dma_start(axs_sb[:], auxs[:])
            nc.scalar.dma_start(axb_sb[:], auxb[:])
            nc.scalar.dma_start(xa1[:, 3 * W:], xt[1][:, 3 * W:])
            nc.scalar.dma_start(wk_sb[:, 4 * 128:], wkb[:, 4 * 128:])

            v_sb = axs_sb[:, 0:NV]
            io_sb = axs_sb[:, NV:NV + 128]
            wd_sb = axb_sb[:, 0:NIN]
            od_sb = axb_sb[:, NIN:2 * NIN]

            def wk_tile(k, c):
                i = WKPOS[(k, c)]
                return wk_sb[:, i * 128:(i + 1) * 128]

            # PE warm-up scratch (vector memset: gpsimd is busy triggering x)
            scr = const.tile([128, 512], BF16, tag="scr")
            nc.vector.memset(scr[:], 0.0)

            # scatter masks from iota while waiting on aux:
            # Lmod[j, m] = 1 iff LCOL[j] % 128 == m
            lmod = const.tile([128, 128], BF16, tag="lmod")
            nc.vector.tensor_scalar(lmod[:], io_sb, v_sb[:, 8:9],
                                    None, op0=ALU.is_equal)
            rmod = const.tile([128, 128], BF16, tag="rmod")
            nc.vector.tensor_scalar(rmod[:], io_sb, v_sb[:, 9:10],
                                    None, op0=ALU.is_equal)

            # ---------- aux compute: dd[j] = sum_i |w[i,d_j] - old[i,d_j]| --
            dch = aux.tile([128, NIN], F32, tag="dch")
            nc.vector.tensor_tensor(dch[:], wd_sb, od_sb, op=ALU.subtract)
            dd = const.tile([128, 1], F32, tag="dd")
            nc.vector.tensor_reduce(
                dd[:], dch[:], axis=mybir.AxisListType.X, op=ALU.add,
                apply_absolute_value=True,
            )
            # active = (dd > THRESHOLD) & ((batch_ctr - indicator) > REF_PERIOD)
            t1 = const.tile([128, 1], F32, tag="t1")
            nc.vector.tensor_tensor(t1[:], v_sb[:, 3:4], v_sb[:, 2:3],
                                    op=ALU.subtract)
            c2 = const.tile([128, 1], F32, tag="c2")
            nc.vector.tensor_scalar(c2[:], t1[:], REF_PERIOD, None, op0=ALU.is_gt)
            c1 = const.tile([128, 1], F32, tag="c1")
            nc.vector.tensor_scalar(c1[:], dd[:], THRESHOLD, None, op0=ALU.is_gt)
            av = const.tile([128, 1], F32, tag="av")
            nc.vector.tensor_tensor(av[:], c1[:], c2[:], op=ALU.mult)
            da = const.tile([128, 1], F32, tag="da")
            nc.vector.tensor_tensor(da[:], dd[:], av[:], op=ALU.mult)
            lf1 = const.tile([128, 1], F32, tag="lf1")
            nc.vector.tensor_tensor(lf1[:], da[:], v_sb[:, 0:1], op=ALU.mult)
            rf1 = const.tile([128, 1], F32, tag="rf1")
            nc.vector.tensor_tensor(rf1[:], da[:], v_sb[:, 1:2], op=ALU.mult)

            # additive scatters (all 4 chunks in one matmul pair), then
            # mult = (1 + L^T lfm1) * (1 + R^T rfm1)
            lfc = const.tile([128, CC], BF16, tag="lfc")
            nc.vector.tensor_scalar(lfc[:], v_sb[:, 10:10 + CC], lf1[:],
                                    None, op0=ALU.mult)
            rfc = const.tile([128, CC], BF16, tag="rfc")
            nc.vector.tensor_scalar(rfc[:], v_sb[:, 14:14 + CC], rf1[:],
                                    None, op0=ALU.mult)
            with tc.tile_pool(name="psx", bufs=2, space="PSUM") as psaux:
                warm = psaux.tile([128, 512], F32, tag="auxps")
                for _ in range(NWARM):
                    nc.tensor.matmul(warm[:], scr[:, :128], scr[:],
                                     start=True, stop=True)
                psl = psaux.tile([128, CC], F32, tag="auxps")
                nc.tensor.matmul(psl[:], lmod[:], lfc[:], start=True, stop=True)
                psr = psaux.tile([128, CC], F32, tag="auxps")
                nc.tensor.matmul(psr[:], rmod[:], rfc[:], start=True, stop=True)
                lsp = const.tile([128, CC], F32, tag="lsp")
                nc.vector.tensor_scalar(lsp[:], psl[:], 1.0, None, op0=ALU.add)
                rsp = const.tile([128, CC], F32, tag="rsp")
                nc.vector.tensor_scalar(rsp[:], psr[:], 1.0, None, op0=ALU.add)
                multm = const.tile([128, CC], F32, tag="multm")
                nc.vector.tensor_tensor(multm[:], lsp[:], rsp[:], op=ALU.mult)
            mult_sb = [multm[:, cc:cc + 1] for cc in range(CC)]

            # ---------- main: y^T = (w^T x^T) scaled+biased+relu ----------
            # Window pairs share each stationary weight across 4 matmuls.
            def evict_act(ps, ob, c, half=None):
                lo = c * W if half in (None, 0) else c * W + 512
                hi = (c + 1) * W if half in (None, 1) else c * W + 512
                plo = 0 if half in (None, 0) else 512
                phi = W if half in (None, 1) else 512
                nc.scalar.activation(
                    ob[:, lo:hi], ps[:, plo:phi], AF.Relu,
                    bias=v_sb[:, 4 + c:5 + c], scale=mult_sb[c])

            def evict_dve(ps, ob, c, half=None):
                lo = c * W if half in (None, 0) else c * W + 512
                hi = (c + 1) * W if half in (None, 1) else c * W + 512
                plo = 0 if half in (None, 0) else 512
                phi = W if half in (None, 1) else 512
                n = hi - lo
                tmp = tpool.tile([128, W], F32, tag="evt")
                nc.vector.tensor_scalar(
                    tmp[:, :n], ps[:, plo:phi], mult_sb[c], v_sb[:, 4 + c:5 + c],
                    op0=ALU.mult, op1=ALU.add)
                nc.vector.tensor_scalar(
                    ob[:, lo:hi], tmp[:, :n], 0.0, None, op0=ALU.max)

            NSW = NWP // 2
            with tc.tile_pool(name="ps", bufs=4, space="PSUM") as pspool:
                for sw in range(NSW):
                    if sw + 1 < NSW:
                        load_xa(2 * (sw + 1))
                        load_xa(2 * (sw + 1) + 1)
                    wpa, wpb = 2 * sw, 2 * sw + 1
                    xaa, xab = xa_tiles[wpa], xa_tiles[wpb]
                    oba = opool.tile([128, CC * W], BF16, tag="ob")
                    obb = opool.tile([128, CC * W], BF16, tag="ob")
                    last = sw == NSW - 1
                    for c in range(CC):
                        psa = pspool.tile([128, W], F32, tag="mps")
                        psb = pspool.tile([128, W], F32, tag="mps")
                        korder = range(KC) if c % 2 == 0 \
                            else range(KC - 1, -1, -1)
                        for ki, k in enumerate(korder):
                            for ps, xa in ((psa, xaa), (psb, xab)):
                                for s in range(W // 512):
                                    nc.tensor.matmul(
                                        ps[:, s * 512:(s + 1) * 512],
                                        wk_tile(k, c),
                                        xa[:, k * W + s * 512: k * W + (s + 1) * 512],
                                        start=(ki == 0), stop=(ki == KC - 1),
                                    )
                        if last and c == CC - 1:
                            # tail: halve evictions so the final DMAs start
                            # one half early, spread over all three queues
                            evict_act(psa, oba, c, half=0)
                            evict_dve(psb, obb, c, half=0)
                            h = c * W + 512
                            nc.sync.dma_start(yt[wpa][:, c * W:h],
                                              oba[:, c * W:h])
                            nc.gpsimd.dma_start(yt[wpb][:, c * W:h],
                                                obb[:, c * W:h])
                            evict_act(psa, oba, c, half=1)
                            evict_dve(psb, obb, c, half=1)
                            nc.scalar.dma_start(yt[wpa][:, h:(c + 1) * W],
                                                oba[:, h:(c + 1) * W])
                            nc.sync.dma_start(yt[wpb][:, h:(c + 1) * W],
                                              obb[:, h:(c + 1) * W])
                            continue
                        # steady state: c0/c1 -> scalar ACT, c2/c3 -> vector
                        # DVE (parallel engines); drain c0/c1 via the scalar
                        # queue, c2 via gpsimd, c3 via sync
                        if c < 2:
                            evict_act(psa, oba, c)
                            evict_act(psb, obb, c)
                            enga = engb = nc.scalar
                        elif c == 2:
                            evict_dve(psa, oba, c)
                            evict_dve(psb, obb, c)
                            enga = engb = nc.gpsimd
                        else:
                            evict_dve(psa, oba, c)
                            evict_dve(psb, obb, c)
                            enga = engb = nc.sync
                        enga.dma_start(yt[wpa][:, c * W:(c + 1) * W],
                                       oba[:, c * W:(c + 1) * W])
                        engb.dma_start(yt[wpb][:, c * W:(c + 1) * W],
                                       obb[:, c * W:(c + 1) * W])

    nc.compile()
    _CACHED_NC = nc
    return nc


LAST_RESULTS = None


def kernel(x, w, b, dop_weights_old, indicator, batch_ctr):
    global LAST_RESULTS
    x = np.asarray(x, dtype=np.float32)
    w = np.ascontiguousarray(np.asarray(w, dtype=np.float32))
    b_arr = np.asarray(b, dtype=np.float32)
    old = np.asarray(dop_weights_old, dtype=np.float32)
    ind = np.asarray(indicator, dtype=np.float32)
    bc_val = float(np.asarray(batch_ctr).item())

    nc = build_nc()

    # replicated (per-core identical) inputs; all reshapes/gathers are pure
    # data marshaling -- every arithmetic op happens on device
    w4 = w.reshape(KC, 128, CC, 128)
    wkb = np.ascontiguousarray(np.concatenate(
        [w4[k, :, c, :] for (k, c) in ORDER], axis=1)).astype(BF16_NP)
    vcols = [LOK10, ROK10, ind.astype(np.float32),
             np.full(128, bc_val, np.float32)]
    vcols += [b_arr[c * 128:(c + 1) * 128] for c in range(CC)]
    vcols += [(LCOL % 128).astype(np.float32), (RCOL % 128).astype(np.float32)]
    vcols += [(LCOL // 128 == cc).astype(np.float32) for cc in range(CC)]
    vcols += [(RCOL // 128 == cc).astype(np.float32) for cc in range(CC)]
    vecs = np.stack(vcols, axis=1).astype(np.float32)
    iot = np.broadcast_to(np.arange(128, dtype=np.float32), (128, 128))
    auxs = np.ascontiguousarray(np.concatenate(
        [vecs, iot], axis=1, dtype=np.float32))
    auxb = np.ascontiguousarray(np.concatenate(
        [w.T[DOP_IDX], old.T[DOP_IDX]], axis=1, dtype=np.float32)
    ).astype(BF16_NP)

    common = dict(wkb=wkb, auxs=auxs, auxb=auxb)

    xbf = x.astype(BF16_NP)
    in_maps = []
    for i in range(N_CORES):
        xs = xbf[i * SHARD:(i + 1) * SHARD]          # [8192, 512]
        xtc = np.ascontiguousarray(
            xs.reshape(NWP, W, KC, 128).transpose(0, 3, 2, 1)
        ).reshape(NWP, 128, KC * W)
        in_maps.append(dict(common, xt=xtc))

    res = run_bass_kernel_spmd(nc, in_maps, core_ids=list(range(N_CORES)))
    LAST_RESULTS = res

    out = np.empty((B, UNITS), np.float32)
    for i in range(N_CORES):
        ytc = res.results[i]["yt"].astype(np.float32).reshape(NWP, 128, CC, W)
        out[i * SHARD:(i + 1) * SHARD] = (
            ytc.transpose(0, 3, 2, 1).reshape(SHARD, UNITS))
    return out


# revision 7
# speedup vs baseline: 1.0825x; 1.0295x over previous
"""DopDense forward: relu(x @ (w * mult) + b) on 8 trn2 NeuronCores.

Key algebra: w_new = w * mult (per-column scaling) commutes with the matmul,
so out = relu((x @ w) * mult[None, :] + b).  We compute y^T tiles (units on
partitions, batch on free axis) so the per-column mult/bias become
per-partition scale/bias of a fused Relu eviction (scalar-engine activation
or a 2-op vector tensor_scalar).

mult is computed on device: dd[j] = sum_i |w[i,d_j] - old[i,d_j]| (vector
engine), gating logic in j-space, then a multiplicative scatter to columns
as mult = (1 + L^T lfm1) * (1 + R^T rfm1) -- left/right target columns are
each unique, and the single collision (column 0) is handled exactly by the
product.  L/R are built on device from an iota constant via is_equal.

Sharding: data-parallel over the batch axis (8192 rows/core); w, dop state
replicated.  The big matmul runs in bf16 and the output is stored in bf16
(upconverted on host), so the kernel is tensor-engine bound (~55us of
matmul issue) with DMA (~18 MB/core) fully overlapped underneath.  DMA
priorities: the first window pair's x chunks lead the sync/gpsimd queues at
k-chunk granularity so the matmul stream starts as soon as data can land;
aux inputs lead the scalar queue so the column-scale vector is ready well
before the first eviction.
"""

import numpy as np
import ml_dtypes


def _install_ntff_shim():
    """The trimmed antenv package in this image lacks axon_hooks, which
    concourse's trace=True path imports unconditionally.  Recreate the hook
    registry (and install the ctypes NTFF hook when available) so tracing
    works whether or not the caller enables it."""
    import sys
    import types
    try:
        import antenv
        import antenv.axon_hooks  # noqa: F401
        return
    except ImportError:
        pass
    try:
        import antenv
    except ImportError:
        return
    mod = types.ModuleType("antenv.axon_hooks")
    holder = [None]
    try:
        from trn_agent_boot.trn_boot import _ntff_profile_via_ctypes
        holder[0] = _ntff_profile_via_ctypes("/opt/axon/libaxon_pjrt.so")
    except Exception:
        pass
    mod.get_axon_ntff_profile_hook = lambda: holder[0]
    mod.set_axon_ntff_profile_hook = lambda h: holder.__setitem__(0, h)
    sys.modules["antenv.axon_hooks"] = mod
    antenv.axon_hooks = mod


_install_ntff_shim()

import concourse.bass as bass
import concourse.mybir as mybir
import concourse.tile as tile
from concourse import bacc
from concourse.bass_utils import run_bass_kernel_spmd

F32 = mybir.dt.float32
BF16 = mybir.dt.bfloat16
AF = mybir.ActivationFunctionType
ALU = mybir.AluOpType
BF16_NP = np.dtype(ml_dtypes.bfloat16)

N_CORES = 8
B = 65536
NIN = 512
UNITS = 512
N_DOP = 128
SHARD = B // N_CORES          # 8192 batch rows per core
W = 1024                      # batch window per psum tile (2 PSUM banks)
NWP = SHARD // W              # 8 windows per core
KC = NIN // 128               # 4 contraction chunks
CC = UNITS // 128             # 4 unit chunks
THRESHOLD = 0.0
REF_PERIOD = 2.0
NWARM = 7                     # PE clock warm-up matmuls

# w chunk (k, c) visit order of the main loop: c outer, k fwd for even c and
# rev for odd c (shared x-chunk locality at c boundaries).  wkb is packed on
# host in this first-use order so a prefix DMA covers the c=0 chunks.
ORDER = []
for _c in range(CC):
    _ks = range(KC) if _c % 2 == 0 else range(KC - 1, -1, -1)
    for _k in _ks:
        ORDER.append((_k, _c))
WKPOS = {kc: i for i, kc in enumerate(ORDER)}

# Static dopaminergic-column index math (mirrors reference.py exactly)
DOP_IDX = np.linspace(1, UNITS - 1, N_DOP, dtype=np.int32)
LEFT_OK = ~np.isin(DOP_IDX - 1, DOP_IDX)
RIGHT_OK = ~np.isin(DOP_IDX + 1, DOP_IDX)
LCOL = (DOP_IDX - 1) % UNITS
RCOL = (DOP_IDX + 1) % UNITS

LOK10 = LEFT_OK.astype(np.float32) * np.float32(10.0 / NIN)
ROK10 = RIGHT_OK.astype(np.float32) * np.float32(10.0 / NIN)

_CACHED_NC = None


def build_nc():
    global _CACHED_NC
    if _CACHED_NC is not None:
        return _CACHED_NC
    nc = bacc.Bacc("TRN2", target_bir_lowering=False, debug=False,
                   num_swdge_queues=2)

    xt = nc.dram_tensor("xt", [NWP, 128, KC * W], BF16, kind="ExternalInput")
    # w chunks packed as [128, ORDER-position * 128 + m] (bf16, stationary)
    wkb = nc.dram_tensor("wkb", [128, KC * CC * 128], BF16, kind="ExternalInput")
    # aux inputs packed into one wide tensor (small-row DMAs are slow):
    # [:, 0:18] = per-partition vectors (lok10, rok10, indicator, batch_ctr,
    # b0..b3, lcol%128, rcol%128, Lchunkmask[4], Rchunkmask[4]),
    # [:, 18:146] = iota rows
    NV = 18
    auxs = nc.dram_tensor("auxs", [128, NV + 128], F32, kind="ExternalInput")
    # dop columns of w^T and old^T, in bf16 (the |w-old| reduction over 512
    # terms is insensitive to bf16 rounding; halves the critical aux DMA)
    auxb = nc.dram_tensor("auxb", [128, 2 * NIN], BF16, kind="ExternalInput")
    # output in bf16 (rel-err budget 2e-2; bf16 adds ~2e-3) -- halves the
    # dominant output DMA traffic. Host upconverts to fp32.
    yt = nc.dram_tensor("yt", [NWP, 128, CC * W], BF16, kind="ExternalOutput")

    with tile.TileContext(nc) as tc:
        with (
            tc.tile_pool(name="const", bufs=1) as const,
            tc.tile_pool(name="aux", bufs=1) as aux,
            tc.tile_pool(name="xa", bufs=4) as xpool,
            tc.tile_pool(name="ob", bufs=5) as opool,
            tc.tile_pool(name="tmp", bufs=2) as tpool,
        ):
            wk_sb = const.tile([128, KC * CC * 128], BF16, tag="wk")
            axs_sb = const.tile([128, NV + 128], F32, tag="axs")
            axb_sb = const.tile([128, 2 * NIN], BF16, tag="axb")
            xa_tiles = {}

            def load_xa(wp):
                xa = xpool.tile([128, KC * W], BF16, tag="xa")
                eng = nc.sync if wp % 2 == 0 else nc.gpsimd
                eng.dma_start(xa[:, :2 * W], xt[wp][:, :2 * W])
                eng.dma_start(xa[:, 2 * W:], xt[wp][:, 2 * W:])
                xa_tiles[wp] = xa

            # ---------- input DMAs: emission order = per-queue priority ----
            # Pair-0 x chunks interleave both windows across the sync and
            # gpsimd queues (k-granular: each chunk unblocks its matmuls as
            # soon as it lands -- deps are per-DMA-instruction).  wkA (the
            # c=0 stationary chunks) leads sync; aux leads scalar.
            xa0 = xpool.tile([128, KC * W], BF16, tag="xa")
            xa1 = xpool.tile([128, KC * W], BF16, tag="xa")
            xa_tiles[0], xa_tiles[1] = xa0, xa1

            def xk(eng, xa, wp, k):
                eng.dma_start(xa[:, k * W:(k + 1) * W],
                              xt[wp][:, k * W:(k + 1) * W])

            nc.sync.dma_start(wk_sb[:, :4 * 128], wkb[:, :4 * 128])
            xk(nc.sync, xa0, 0, 0)
            xk(nc.gpsimd, xa1, 1, 0)
            nc.scalar.dma_start(axb_sb[:], auxb[:])
            xk(nc.gpsimd, xa0, 0, 1)
            xk(nc.sync, xa1, 1, 1)
            nc.scalar.dma_start(axs_sb[:], auxs[:])
            xk(nc.sync, xa0, 0, 2)
            xk(nc.gpsimd, xa1, 1, 2)
            nc.scalar.dma_start(wk_sb[:, 4 * 128:], wkb[:, 4 * 128:])
            xk(nc.gpsimd, xa0, 0, 3)
            xk(nc.sync, xa1, 1, 3)

            v_sb = axs_sb[:, 0:NV]
            io_sb = axs_sb[:, NV:NV + 128]
            wd_sb = axb_sb[:, 0:NIN]
            od_sb = axb_sb[:, NIN:2 * NIN]

            def wk_tile(k, c):
                i = WKPOS[(k, c)]
                return wk_sb[:, i * 128:(i + 1) * 128]

            # PE warm-up scratch (vector memset: gpsimd is busy triggering x)
            scr = const.tile([128, 512], BF16, tag="scr")
            nc.vector.memset(scr[:], 0.0)

            # scatter masks from iota while waiting on aux:
            # Lmod[j, m] = 1 iff LCOL[j] % 128 == m
            lmod = const.tile([128, 128], BF16, tag="lmod")
            nc.vector.tensor_scalar(lmod[:], io_sb, v_sb[:, 8:9],
                                    None, op0=ALU.is_equal)
            rmod = const.tile([128, 128], BF16, tag="rmod")
            nc.vector.tensor_scalar(rmod[:], io_sb, v_sb[:, 9:10],
                                    None, op0=ALU.is_equal)

            # ---------- aux compute: dd[j] = sum_i |w[i,d_j] - old[i,d_j]| --
            dch = aux.tile([128, NIN], F32, tag="dch")
            nc.vector.tensor_tensor(dch[:], wd_sb, od_sb, op=ALU.subtract)
            dd = const.tile([128, 1], F32, tag="dd")
            nc.vector.tensor_reduce(
                dd[:], dch[:], axis=mybir.AxisListType.X, op=ALU.add,
                apply_absolute_value=True,
            )
            # active = (dd > THRESHOLD) & ((batch_ctr - indicator) > REF_PERIOD)
            t1 = const.tile([128, 1], F32, tag="t1")
            nc.vector.tensor_tensor(t1[:], v_sb[:, 3:4], v_sb[:, 2:3],
                                    op=ALU.subtract)
            c2 = const.tile([128, 1], F32, tag="c2")
            nc.vector.tensor_scalar(c2[:], t1[:], REF_PERIOD, None, op0=ALU.is_gt)
            c1 = const.tile([128, 1], F32, tag="c1")
            nc.vector.tensor_scalar(c1[:], dd[:], THRESHOLD, None, op0=ALU.is_gt)
            av = const.tile([128, 1], F32, tag="av")
            nc.vector.tensor_tensor(av[:], c1[:], c2[:], op=ALU.mult)
            da = const.tile([128, 1], F32, tag="da")
            nc.vector.tensor_tensor(da[:], dd[:], av[:], op=ALU.mult)
            lf1 = const.tile([128, 1], F32, tag="lf1")
            nc.vector.tensor_tensor(lf1[:], da[:], v_sb[:, 0:1], op=ALU.mult)
            rf1 = const.tile([128, 1], F32, tag="rf1")
            nc.vector.tensor_tensor(rf1[:], da[:], v_sb[:, 1:2], op=ALU.mult)

            # additive scatters (all 4 chunks in one matmul pair), then
            # mult = (1 + L^T lfm1) * (1 + R^T rfm1)
            lfc = const.tile([128, CC], BF16, tag="lfc")
            nc.vector.tensor_scalar(lfc[:], v_sb[:, 10:10 + CC], lf1[:],
                                    None, op0=ALU.mult)
            rfc = const.tile([128, CC], BF16, tag="rfc")
            nc.vector.tensor_scalar(rfc[:], v_sb[:, 14:14 + CC], rf1[:],
                                    None, op0=ALU.mult)
            # warm / scatter psums live in the MAIN psum pool (a separate
            # pool's release would serialize the whole aux chain before the
            # first main matmul via the pool-boundary dependency)
            multm = const.tile([128, CC], F32, tag="multm")
            mult_sb = [multm[:, cc:cc + 1] for cc in range(CC)]

            # ---------- main: y^T = (w^T x^T) scaled+biased+relu ----------
            # Window pairs share each stationary weight across 4 matmuls.
            def evict_act(ps, ob, c, half=None):
                lo = c * W if half in (None, 0) else c * W + 512
                hi = (c + 1) * W if half in (None, 1) else c * W + 512
                plo = 0 if half in (None, 0) else 512
                phi = W if half in (None, 1) else 512
                nc.scalar.activation(
                    ob[:, lo:hi], ps[:, plo:phi], AF.Relu,
                    bias=v_sb[:, 4 + c:5 + c], scale=mult_sb[c])

            def evict_dve(ps, ob, c, half=None):
                lo = c * W if half in (None, 0) else c * W + 512
                hi = (c + 1) * W if half in (None, 1) else c * W + 512
                plo = 0 if half in (None, 0) else 512
                phi = W if half in (None, 1) else 512
                n = hi - lo
                tmp = tpool.tile([128, W], F32, tag="evt")
                nc.vector.tensor_scalar(
                    tmp[:, :n], ps[:, plo:phi], mult_sb[c], v_sb[:, 4 + c:5 + c],
                    op0=ALU.mult, op1=ALU.add)
                nc.vector.tensor_scalar(
                    ob[:, lo:hi], tmp[:, :n], 0.0, None, op0=ALU.max)

            NSW = NWP // 2
            with tc.tile_pool(name="ps", bufs=4, space="PSUM") as pspool:
                # PE clock warm-up burst bridges the gap until the first x
                # chunk lands, so the HAM clock gate opens just as the real
                # stream begins
                warm = pspool.tile([128, W], F32, tag="mps")
                for _ in range(NWARM):
                    nc.tensor.matmul(warm[:, :512], scr[:, :128], scr[:],
                                     start=True, stop=True)

                def aux_scatter():
                    # scatter matmuls injected mid-stream (pair 0, between
                    # c0's k2 and k3 blocks): by then lfc/rfc are ready, and
                    # multm lands just before the first eviction needs it
                    psl = pspool.tile([128, W], F32, tag="mps")
                    nc.tensor.matmul(psl[:, :CC], lmod[:], lfc[:],
                                     start=True, stop=True)
                    psr = pspool.tile([128, W], F32, tag="mps")
                    nc.tensor.matmul(psr[:, :CC], rmod[:], rfc[:],
                                     start=True, stop=True)
                    lsp = const.tile([128, CC], F32, tag="lsp")
                    nc.vector.tensor_scalar(lsp[:], psl[:, :CC], 1.0, None,
                                            op0=ALU.add)
                    rsp = const.tile([128, CC], F32, tag="rsp")
                    nc.vector.tensor_scalar(rsp[:], psr[:, :CC], 1.0, None,
                                            op0=ALU.add)
                    nc.vector.tensor_tensor(multm[:], lsp[:], rsp[:],
                                            op=ALU.mult)

                for sw in range(NSW):
                    if sw + 1 < NSW:
                        load_xa(2 * (sw + 1))
                        load_xa(2 * (sw + 1) + 1)
                    wpa, wpb = 2 * sw, 2 * sw + 1
                    xaa, xab = xa_tiles[wpa], xa_tiles[wpb]
                    oba = opool.tile([128, CC * W], BF16, tag="ob")
                    obb = opool.tile([128, CC * W], BF16, tag="ob")
                    last = sw == NSW - 1
                    for c in range(CC):
                        psa = pspool.tile([128, W], F32, tag="mps")
                        psb = pspool.tile([128, W], F32, tag="mps")
                        korder = range(KC) if c % 2 == 0 \
                            else range(KC - 1, -1, -1)
                        for ki, k in enumerate(korder):
                            if sw == 0 and c == 0 and ki == KC - 1:
                                aux_scatter()
                            for ps, xa in ((psa, xaa), (psb, xab)):
                                for s in range(W // 512):
                                    nc.tensor.matmul(
                                        ps[:, s * 512:(s + 1) * 512],
                                        wk_tile(k, c),
                                        xa[:, k * W + s * 512: k * W + (s + 1) * 512],
                                        start=(ki == 0), stop=(ki == KC - 1),
                                    )
                        if last and c == CC - 1:
                            # tail: halve evictions so the final DMAs start
                            # one half early; no gpsimd here (its SWDGE
                            # drain instruction costs ~4us at kernel end)
                            evict_act(psa, oba, c, half=0)
                            evict_dve(psb, obb, c, half=0)
                            h = c * W + 512
                            nc.sync.dma_start(yt[wpa][:, c * W:h],
                                              oba[:, c * W:h])
                            nc.scalar.dma_start(yt[wpb][:, c * W:h],
                                                obb[:, c * W:h])
                            evict_act(psa, oba, c, half=1)
                            evict_dve(psb, obb, c, half=1)
                            nc.scalar.dma_start(yt[wpa][:, h:(c + 1) * W],
                                                oba[:, h:(c + 1) * W])
                            nc.sync.dma_start(yt[wpb][:, h:(c + 1) * W],
                                              obb[:, h:(c + 1) * W])
                            continue
                        if sw == 0 and c == 0:
                            # pair-0 c0 banks gate c1 through the 4-deep
                            # psum rotation (warm/psl/psr shift it); split
                            # each eviction across both engines to free the
                            # banks ~2x sooner
                            evict_act(psa, oba, c, half=0)
                            evict_dve(psa, oba, c, half=1)
                            evict_act(psb, obb, c, half=1)
                            evict_dve(psb, obb, c, half=0)
                            enga = engb = nc.scalar
                        elif c < 2:
                            # steady state: c0/c1 -> scalar ACT, c2/c3 ->
                            # vector DVE (parallel engines)
                            evict_act(psa, oba, c)
                            evict_act(psb, obb, c)
                            enga = engb = nc.scalar
                        elif c == 2:
                            evict_dve(psa, oba, c)
                            evict_dve(psb, obb, c)
                            # keep the slow-draining SWDGE queue away from
                            # the kernel end
                            enga = engb = nc.gpsimd if not last else nc.scalar
                        else:
                            evict_dve(psa, oba, c)
                            evict_dve(psb, obb, c)
                            enga = engb = nc.sync
                        enga.dma_start(yt[wpa][:, c * W:(c + 1) * W],
                                       oba[:, c * W:(c + 1) * W])
                        engb.dma_start(yt[wpb][:, c * W:(c + 1) * W],
                                       obb[:, c * W:(c + 1) * W])

    nc.compile()
    _CACHED_NC = nc
    return nc


LAST_RESULTS = None


def kernel(x, w, b, dop_weights_old, indicator, batch_ctr):
    global LAST_RESULTS
    x = np.asarray(x, dtype=np.float32)
    w = np.ascontiguousarray(np.asarray(w, dtype=np.float32))
    b_arr = np.asarray(b, dtype=np.float32)
    old = np.asarray(dop_weights_old, dtype=np.float32)
    ind = np.asarray(indicator, dtype=np.float32)
    bc_val = float(np.asarray(batch_ctr).item())

    nc = build_nc()

    # replicated (per-core identical) inputs; all reshapes/gathers are pure
    # data marshaling -- every arithmetic op happens on device
    w4 = w.reshape(KC, 128, CC, 128)
    wkb = np.ascontiguousarray(np.concatenate(
        [w4[k, :, c, :] for (k, c) in ORDER], axis=1)).astype(BF16_NP)
    vcols = [LOK10, ROK10, ind.astype(np.float32),
             np.full(128, bc_val, np.float32)]
    vcols += [b_arr[c * 128:(c + 1) * 128] for c in range(CC)]
    vcols += [(LCOL % 128).astype(np.float32), (RCOL % 128).astype(np.float32)]
    vcols += [(LCOL // 128 == cc).astype(np.float32) for cc in range(CC)]
    vcols += [(RCOL // 128 == cc).astype(np.float32) for cc in range(CC)]
    vecs = np.stack(vcols, axis=1).astype(np.float32)
    iot = np.broadcast_to(np.arange(128, dtype=np.float32), (128, 128))
    auxs = np.ascontiguousarray(np.concatenate(
        [vecs, iot], axis=1, dtype=np.float32))
    auxb = np.ascontiguousarray(np.concatenate(
        [w.T[DOP_IDX], old.T[DOP_IDX]], axis=1, dtype=np.float32)
    ).astype(BF16_NP)

    common = dict(wkb=wkb, auxs=auxs, auxb=auxb)

    xbf = x.astype(BF16_NP)
    in_maps = []
    for i in range(N_CORES):
        xs = xbf[i * SHARD:(i + 1) * SHARD]          # [8192, 512]
        xtc = np.ascontiguousarray(
            xs.reshape(NWP, W, KC, 128).transpose(0, 3, 2, 1)
        ).reshape(NWP, 128, KC * W)
        in_maps.append(dict(common, xt=xtc))

    res = run_bass_kernel_spmd(nc, in_maps, core_ids=list(range(N_CORES)))
    LAST_RESULTS = res

    out = np.empty((B, UNITS), np.float32)
    for i in range(N_CORES):
        ytc = res.results[i]["yt"].astype(np.float32).reshape(NWP, 128, CC, W)
        out[i * SHARD:(i + 1) * SHARD] = (
            ytc.transpose(0, 3, 2, 1).reshape(SHARD, UNITS))
    return out
